# revision 4
# baseline (speedup 1.0000x reference)
"""Trainium2 Bass kernel for the 2-layer LSTM language-model problem.

Strategy (8 NeuronCores, SPMD):
  - Tensor-parallel over the 4*NN gate dimension: core k owns hidden chunk k
    (128 of 1024 hidden units) of BOTH LSTM layers; the per-step hidden state
    is re-assembled with an AllGather of transposed h-chunks.
  - Phase A: s0x[t,b,:] = inputs @ (emb_matrix @ W0x_chunk) + b0_chunk
    precomputed for all timesteps (associativity fuses the embedding).
  - Phase B: 200 recurrent steps. Per step and layer: 8..16 accumulating
    matmuls (activations stationary, weights streaming from SBUF), sigmoid/
    tanh on ScalarE, elementwise on DVE, PE transpose of the new h chunk,
    AllGather. Gathered h1T is also scattered (per-core rotated) into a DRAM
    history buffer so phase C can read its row shard at static addresses.
  - Phase C: output MLP on a 25-timestep shard per core:
    z1T = relu(ow0.T @ hsT + b0) computed transposed (weights stationary),
    then logits rows = z1 @ ow1 + b1 (activations stationary).
Host side only reshapes/slices numpy inputs and concatenates the 8 output
row-shards.
"""

import sys
import os

for _p in ("/opt/trn_rl_repo", "/root/.axon_site/_ro/trn_rl_repo"):
    if os.path.isdir(_p) and _p not in sys.path:
        sys.path.insert(0, _p)

import numpy as np

import concourse.bass as bass
import concourse.mybir as mybir
import concourse.tile as tile
from concourse import bacc
from concourse.bass_utils import run_bass_kernel_spmd
from concourse.masks import make_identity

F32 = mybir.dt.float32
I32 = mybir.dt.int32
AF = mybir.ActivationFunctionType

# problem shapes (hardcoded per contract)
T, B, V, E, NN, ON = 200, 64, 256, 512, 1024, 1024
N_CORES = 8
CH = NN // N_CORES          # 128 hidden units per core
NG = 4 * CH                 # 512 gate columns per core
KT0 = NN // 128             # 8 k-tiles for h-part contractions
VT = V // 128               # 2 v-tiles
ET = E // 128               # 4 e-tiles
MT = ON // 128              # 8 hid2 tiles

_CACHE = {}


def _build_program(nsteps: int):
    """Build the SPMD Bass program (identical for every core)."""
    nblocks = (nsteps * B) // 512          # phase-A row blocks of 512
    ts_shard = nsteps // N_CORES           # phase-C timesteps per core
    rows = ts_shard * B

    nc = bacc.Bacc("TRN2", target_bir_lowering=False, debug=False,
                   num_devices=N_CORES)

    def din(name, shape, dt=F32):
        return nc.dram_tensor(name, shape, dt, kind="ExternalInput").ap()

    inputsT = din("inputsT", [V, nsteps * B])       # replicated
    emT = din("emT", [E, V])                        # emb_matrix.T, replicated
    w0x = din("w0x", [E, NG])                       # lstm_w0[:E, cols_k]
    w0h = din("w0h", [NN, NG])                      # lstm_w0[E:, cols_k]
    w1 = din("w1", [2 * NN, NG])                    # lstm_w1[:, cols_k]
    b0c = din("b0c", [128, NG])                     # b0[cols_k] bcast rows
    b1c = din("b1c", [B, NG])                       # b1[cols_k] bcast rows
    h0T_i = din("h0T", [NN, B])                     # h0.T, replicated
    h1T_i = din("h1T", [NN, B])
    c0_i = din("c0c", [B, CH])                      # c0[:, chunk_k]
    c1_i = din("c1c", [B, CH])
    ow0 = din("ow0", [NN, ON])                      # out_w0, replicated
    ob0 = din("ob0", [ON, 1])                       # out_b0 column, replicated
    ow1 = din("ow1", [ON, V])                       # out_w1, replicated
    ob1 = din("ob1", [128, V])                      # out_b1 bcast, replicated
    scat = din("scat", [128, nsteps], I32)          # history scatter rows

    logits_out = nc.dram_tensor("logits", [rows, V], F32,
                                kind="ExternalOutput").ap()

    with tile.TileContext(nc) as tc:
        with tc.tile_pool(name="dram", bufs=1, space="DRAM") as dram, \
             tc.tile_pool(name="const", bufs=1) as const:
            s0x_d = dram.tile([nsteps * B, NG], F32)
            hist_d = dram.tile([nsteps * 128, NG], F32)
            bounce0 = dram.tile([CH, B], F32)
            bounce1 = dram.tile([CH, B], F32)
            gath0 = dram.tile([NN, B], F32)
            gath1 = dram.tile([NN, B], F32)

            ident = const.tile([B, B], F32)
            make_identity(nc, ident[:])

            # ---------------- phase A: s0x precompute ----------------
            with tc.tile_pool(name="pa", bufs=2) as pa, \
                 tc.tile_pool(name="pa_ps", bufs=3, space="PSUM") as pa_ps:
                emT_s = pa.tile([128, ET * V], F32, tag="emTs")
                nc.sync.dma_start(
                    emT_s[:].rearrange("p (k v) -> p k v", k=ET),
                    emT.rearrange("(k p) v -> p k v", p=128))
                wx_s = pa.tile([128, ET * NG], F32, tag="wxs")
                nc.sync.dma_start(
                    wx_s[:].rearrange("p (k n) -> p k n", k=ET),
                    w0x.rearrange("(k p) n -> p k n", p=128))
                b0_s = pa.tile([128, NG], F32, tag="b0s")
                nc.sync.dma_start(b0_s[:], b0c[:])

                # W_eff [V, NG] = emb @ W0x_chunk
                we_s = pa.tile([128, VT * NG], F32, tag="wes")
                for mm in range(VT):
                    ps = pa_ps.tile([128, NG], F32, tag="we_ps")
                    for kk in range(ET):
                        nc.tensor.matmul(
                            ps[:],
                            emT_s[:, kk * V + mm * 128:
                                  kk * V + (mm + 1) * 128],
                            wx_s[:, kk * NG:(kk + 1) * NG],
                            start=(kk == 0), stop=(kk == ET - 1))
                    nc.vector.tensor_copy(
                        we_s[:, mm * NG:(mm + 1) * NG], ps[:])

                for bi in range(nblocks):
                    r0 = bi * 512
                    inT = pa.tile([128, VT * 512], F32, tag="inT")
                    nc.sync.dma_start(
                        inT[:].rearrange("p (k r) -> p k r", k=VT),
                        inputsT[:, r0:r0 + 512].rearrange(
                            "(k p) r -> p k r", p=128))
                    for mt in range(4):
                        ps = pa_ps.tile([128, NG], F32, tag="sx_ps")
                        for kk in range(VT):
                            nc.tensor.matmul(
                                ps[:],
                                inT[:, kk * 512 + mt * 128:
                                    kk * 512 + (mt + 1) * 128],
                                we_s[:, kk * NG:(kk + 1) * NG],
                                start=(kk == 0), stop=(kk == VT - 1))
                        sx_sb = pa.tile([128, NG], F32, tag="sx_sb")
                        nc.vector.tensor_add(sx_sb[:], ps[:], b0_s[:])
                        nc.sync.dma_start(
                            s0x_d[r0 + mt * 128: r0 + (mt + 1) * 128, :],
                            sx_sb[:])

            # ---------------- phase B: recurrence ----------------
            with tc.tile_pool(name="pb_w", bufs=1) as pb_w, \
                 tc.tile_pool(name="pb_g", bufs=2) as pb_g, \
                 tc.tile_pool(name="pb_ps", bufs=2, space="PSUM") as pb_ps, \
                 tc.tile_pool(name="pb_tps", bufs=2, space="PSUM") as pb_tps, \
                 tc.tile_pool(name="pb_sx", bufs=3) as pb_sx, \
                 tc.tile_pool(name="pb_wk", bufs=3) as pb_wk:

                w0h_s = pb_w.tile([128, KT0 * NG], F32)
                nc.sync.dma_start(
                    w0h_s[:].rearrange("p (k n) -> p k n", k=KT0),
                    w0h.rearrange("(k p) n -> p k n", p=128))
                w1_s = pb_w.tile([128, 2 * KT0 * NG], F32)
                nc.sync.dma_start(
                    w1_s[:].rearrange("p (k n) -> p k n", k=2 * KT0),
                    w1.rearrange("(k p) n -> p k n", p=128))
                b1_s = pb_w.tile([B, NG], F32)
                nc.sync.dma_start(b1_s[:], b1c[:])
                offs_s = pb_w.tile([128, nsteps], I32)
                nc.sync.dma_start(offs_s[:], scat[:])
                cst = pb_w.tile([B, 2 * CH], F32)   # c0 | c1, updated in place
                nc.sync.dma_start(cst[:, 0:CH], c0_i[:])
                nc.sync.dma_start(cst[:, CH:2 * CH], c1_i[:])

                G0 = pb_g.tile([128, KT0 * B], F32, tag="G0")
                nc.sync.dma_start(
                    G0[:].rearrange("p (k b) -> p k b", k=KT0),
                    h0T_i.rearrange("(k p) b -> p k b", p=128))
                G1 = pb_g.tile([128, KT0 * B], F32, tag="G1")
                nc.sync.dma_start(
                    G1[:].rearrange("p (k b) -> p k b", k=KT0),
                    h1T_i.rearrange("(k p) b -> p k b", p=128))

                def lstm_chain(gates, c_sl, tag):
                    """gates [B, NG] pre-activation; updates c slice in
                    place, returns h_new [B, CH] tile."""
                    act = pb_wk.tile([B, NG], F32, tag=f"act{tag}")
                    nc.scalar.activation(act[:, 0:3 * CH],
                                         gates[:, 0:3 * CH], AF.Sigmoid)
                    nc.scalar.activation(act[:, 3 * CH:NG],
                                         gates[:, 3 * CH:NG], AF.Tanh)
                    t1 = pb_wk.tile([B, CH], F32, tag=f"t1{tag}")
                    nc.vector.tensor_mul(t1[:], act[:, 0:CH], c_sl)
                    t2 = pb_wk.tile([B, CH], F32, tag=f"t2{tag}")
                    nc.vector.tensor_mul(t2[:], act[:, CH:2 * CH],
                                         act[:, 3 * CH:NG])
                    nc.vector.tensor_add(c_sl, t1[:], t2[:])
                    thc = pb_wk.tile([B, CH], F32, tag=f"thc{tag}")
                    nc.scalar.activation(thc[:], c_sl, AF.Tanh)
                    h_new = pb_wk.tile([B, CH], F32, tag=f"h{tag}")
                    nc.vector.tensor_mul(h_new[:], act[:, 2 * CH:3 * CH],
                                         thc[:])
                    return h_new

                rg = [list(range(N_CORES))]
                for t in range(nsteps):
                    # ---- layer 0 matmuls ----
                    sx_t = pb_sx.tile([B, NG], F32, tag="sx")
                    nc.sync.dma_start(sx_t[:], s0x_d[t * B:(t + 1) * B, :])
                    ps0 = pb_ps.tile([B, NG], F32, tag="ps0")
                    for kk in range(KT0):
                        nc.tensor.matmul(
                            ps0[:], G0[:, kk * B:(kk + 1) * B],
                            w0h_s[:, kk * NG:(kk + 1) * NG],
                            start=(kk == 0), stop=(kk == KT0 - 1))
                    # layer-1 h-part immediately after: PE stays busy while
                    # the layer-0 chain runs on ACT/DVE
                    ps1 = pb_ps.tile([B, NG], F32, tag="ps1")
                    for kk in range(KT0):
                        nc.tensor.matmul(
                            ps1[:], G1[:, kk * B:(kk + 1) * B],
                            w1_s[:, (KT0 + kk) * NG:(KT0 + kk + 1) * NG],
                            start=(kk == 0), stop=False)

                    g0t = pb_wk.tile([B, NG], F32, tag="g0t")
                    nc.vector.tensor_add(g0t[:], ps0[:], sx_t[:])
                    h0n = lstm_chain(g0t, cst[:, 0:CH], "0")

                    tps0 = pb_tps.tile([CH, B], F32, tag="tps0")
                    nc.tensor.transpose(tps0[:], h0n[:], ident[:])
                    tsb0 = pb_wk.tile([CH, B], F32, tag="tsb0")
                    nc.vector.tensor_copy(tsb0[:], tps0[:])
                    nc.sync.dma_start(bounce0[:], tsb0[:])
                    nc.gpsimd.collective_compute(
                        "AllGather", mybir.AluOpType.bypass,
                        replica_groups=rg,
                        ins=[bounce0.opt()], outs=[gath0.opt()])
                    G0 = pb_g.tile([128, KT0 * B], F32, tag="G0")
                    nc.sync.dma_start(
                        G0[:].rearrange("p (k b) -> p k b", k=KT0),
                        gath0.rearrange("(k p) b -> p k b", p=128))

                    # ---- layer 1 x-part (needs gathered h0n) ----
                    for kk in range(KT0):
                        nc.tensor.matmul(
                            ps1[:], G0[:, kk * B:(kk + 1) * B],
                            w1_s[:, kk * NG:(kk + 1) * NG],
                            start=False, stop=(kk == KT0 - 1))
                    g1t = pb_wk.tile([B, NG], F32, tag="g1t")
                    nc.vector.tensor_add(g1t[:], ps1[:], b1_s[:])
                    h1n = lstm_chain(g1t, cst[:, CH:2 * CH], "1")

                    tps1 = pb_tps.tile([CH, B], F32, tag="tps1")
                    nc.tensor.transpose(tps1[:], h1n[:], ident[:])
                    tsb1 = pb_wk.tile([CH, B], F32, tag="tsb1")
                    nc.vector.tensor_copy(tsb1[:], tps1[:])
                    nc.sync.dma_start(bounce1[:], tsb1[:])
                    nc.gpsimd.collective_compute(
                        "AllGather", mybir.AluOpType.bypass,
                        replica_groups=rg,
                        ins=[bounce1.opt()], outs=[gath1.opt()])
                    G1 = pb_g.tile([128, KT0 * B], F32, tag="G1")
                    nc.sync.dma_start(
                        G1[:].rearrange("p (k b) -> p k b", k=KT0),
                        gath1.rearrange("(k p) b -> p k b", p=128))

                    # store h1T into the (per-core rotated) history
                    nc.gpsimd.indirect_dma_start(
                        out=hist_d[:],
                        out_offset=bass.IndirectOffsetOnAxis(
                            ap=offs_s[:, t:t + 1], axis=0),
                        in_=G1[:],
                        in_offset=None)

            # ---------------- phase C: output MLP on row shard ----------
            with tc.tile_pool(name="pc", bufs=2) as pc, \
                 tc.tile_pool(name="pc_ps", bufs=3, space="PSUM") as pc_ps, \
                 tc.tile_pool(name="pc_z", bufs=1) as pc_z:
                hsT = pc_z.tile([128, ts_shard * NG], F32)  # slot j: h1T(t_j)
                nc.sync.dma_start(
                    hsT[:].rearrange("p (j n) -> p j n", j=ts_shard),
                    hist_d[0:ts_shard * 128, :].rearrange(
                        "(j p) n -> p j n", p=128))
                ow0_s = pc_z.tile([128, KT0 * ON], F32)
                nc.sync.dma_start(
                    ow0_s[:].rearrange("p (k n) -> p k n", k=KT0),
                    ow0.rearrange("(k p) n -> p k n", p=128))
                ob0_s = pc_z.tile([128, MT], F32)
                nc.sync.dma_start(
                    ob0_s[:].rearrange("p (m o) -> p m o", o=1),
                    ob0.rearrange("(m p) o -> p m o", p=128))
                ow1_s = pc_z.tile([128, MT * V], F32)
                nc.sync.dma_start(
                    ow1_s[:].rearrange("p (k n) -> p k n", k=MT),
                    ow1.rearrange("(k p) n -> p k n", p=128))
                ob1_s = pc_z.tile([128, V], F32)
                nc.sync.dma_start(ob1_s[:], ob1[:])

                z1 = pc_z.tile([128, MT * rows], F32)       # z1T slots
                hsT_v = hsT[:].rearrange("p (j n) -> p j n", j=ts_shard)
                rgs = []
                j0 = 0
                while j0 < ts_shard:                        # 8-step groups
                    jn = min(8, ts_shard - j0)
                    rgs.append((j0, jn))
                    j0 += jn
                for m in range(MT):
                    for (j0, jn) in rgs:
                        ps = pc_ps.tile([128, 512], F32, tag="z_ps")
                        psv = ps[:, 0:jn * B].rearrange(
                            "q (j b) -> q j b", j=jn)
                        for kk in range(KT0):
                            nc.tensor.matmul(
                                psv,
                                ow0_s[:, kk * ON + m * 128:
                                      kk * ON + (m + 1) * 128],
                                hsT_v[:, j0:j0 + jn,
                                      kk * B:(kk + 1) * B],
                                start=(kk == 0), stop=(kk == KT0 - 1))
                        nc.scalar.activation(
                            z1[:, m * rows + j0 * B:
                               m * rows + (j0 + jn) * B],
                            ps[:, 0:jn * B], AF.Relu,
                            bias=ob0_s[:, m:m + 1])

                rt0 = 0
                while rt0 < rows:
                    rn = min(128, rows - rt0)
                    ps = pc_ps.tile([128, V], F32, tag="lg_ps")
                    for m in range(MT):
                        nc.tensor.matmul(
                            ps[0:rn, :],
                            z1[:, m * rows + rt0: m * rows + rt0 + rn],
                            ow1_s[:, m * V:(m + 1) * V],
                            start=(m == 0), stop=(m == MT - 1))
                    lg = pc.tile([128, V], F32, tag="lg_sb")
                    nc.vector.tensor_add(lg[0:rn, :], ps[0:rn, :],
                                         ob1_s[0:rn, :])
                    nc.sync.dma_start(logits_out[rt0:rt0 + rn, :],
                                      lg[0:rn, :])
                    rt0 += rn

    nc.compile()
    return nc


def _prep_in_maps(inputs, nsteps):
    """Slice/transpose numpy inputs into per-core input maps."""
    x = np.ascontiguousarray(inputs["inputs"], dtype=np.float32)
    x = x.reshape(nsteps * B, V)
    inputsT = np.ascontiguousarray(x.T)                       # [V, T*B]
    emT = np.ascontiguousarray(
        np.asarray(inputs["emb_matrix"], dtype=np.float32).T)
    w0 = np.asarray(inputs["lstm_w0"], dtype=np.float32)
    w1 = np.asarray(inputs["lstm_w1"], dtype=np.float32)
    b0 = np.asarray(inputs["lstm_b0"], dtype=np.float32)
    b1 = np.asarray(inputs["lstm_b1"], dtype=np.float32)
    h0 = np.asarray(inputs["h0"], dtype=np.float32)
    c0 = np.asarray(inputs["c0"], dtype=np.float32)
    h1 = np.asarray(inputs["h1"], dtype=np.float32)
    c1 = np.asarray(inputs["c1"], dtype=np.float32)
    ow0 = np.ascontiguousarray(inputs["out_w0"], dtype=np.float32)
    ob0 = np.ascontiguousarray(
        np.asarray(inputs["out_b0"], dtype=np.float32).reshape(ON, 1))
    ow1 = np.ascontiguousarray(inputs["out_w1"], dtype=np.float32)
    ob1 = np.ascontiguousarray(
        np.broadcast_to(inputs["out_b1"], (128, V)), dtype=np.float32)
    h0T = np.ascontiguousarray(h0.T)
    h1T = np.ascontiguousarray(h1.T)

    ts_shard = nsteps // N_CORES
    in_maps = []
    for k in range(N_CORES):
        cols = np.concatenate([
            np.arange(g * NN + k * CH, g * NN + (k + 1) * CH)
            for g in range(4)])
        rot = (np.arange(nsteps) - ts_shard * k) % nsteps
        p = np.arange(128)
        scat_rows = np.ascontiguousarray(
            (rot[None, :] * 128 + p[:, None]).astype(np.int32))
        in_maps.append({
            "inputsT": inputsT,
            "emT": emT,
            "w0x": np.ascontiguousarray(w0[:E, cols]),
            "w0h": np.ascontiguousarray(w0[E:, cols]),
            "w1": np.ascontiguousarray(w1[:, cols]),
            "b0c": np.ascontiguousarray(
                np.broadcast_to(b0[cols], (128, NG))),
            "b1c": np.ascontiguousarray(
                np.broadcast_to(b1[cols], (B, NG))),
            "h0T": h0T,
            "h1T": h1T,
            "c0c": np.ascontiguousarray(c0[:, k * CH:(k + 1) * CH]),
            "c1c": np.ascontiguousarray(c1[:, k * CH:(k + 1) * CH]),
            "ow0": ow0,
            "ob0": ob0,
            "ow1": ow1,
            "ob1": ob1,
            "scat": scat_rows,
        })
    return in_maps


def kernel(**inputs):
    nsteps = inputs["inputs"].shape[0]
    if nsteps not in _CACHE:
        _CACHE[nsteps] = _build_program(nsteps)
    nc = _CACHE[nsteps]
    in_maps = _prep_in_maps(inputs, nsteps)
    res = run_bass_kernel_spmd(nc, in_maps, list(range(N_CORES)))
    logits = np.concatenate(
        [res.results[k]["logits"] for k in range(N_CORES)], axis=0)
    return logits
